# revision 25
# baseline (speedup 1.0000x reference)
"""Trainium2 Bass kernel for ContinuousREWAEncoder:
    out = FWHT(x @ W^T)/sqrt(32) + 0.01*normal(key=42)

Math folding: FWHT is linear => out = x @ (H @ W / sqrt(32))^T + noise.
The noise uses a fixed PRNG key, so it is a deterministic constant computed
on host (with the same jax op/backend as the reference) and streamed in.

Sharding: pure data parallel over tokens (B*N = 32768 -> 4096/core on 8
cores). W_eff is replicated. Each x shard is pre-tiled on host so the
contraction dim D lies on SBUF partitions and every DMA is one contiguous
run per partition. The device does a single streaming GEMM:
psum[32, t] += Wc[128,32]^T @ xT[128, t] accumulated over 8 d-chunks, with
the noise added during PSUM evacuation by the DVE, and the output stored
transposed [32, tok] (un-transposed on host).
"""

import math

import numpy as np

import concourse.bass as bass
import concourse.tile as tile
from concourse import bacc, mybir
from concourse.bass_utils import run_bass_kernel_spmd

B, N, D, M = 4, 8192, 1024, 32
NOISE_STD = 0.01
N_CORES = 8
TOK_TOTAL = B * N              # 32768
TOK = TOK_TOTAL // N_CORES     # 4096 tokens per core
BLK = 512                      # tokens per PSUM bank ([32, 512] fp32 = 1 bank)
NBLK = TOK // BLK              # 8 -> exactly the 8 PSUM banks
KC = D // 128                  # 8 contraction chunks
KH = KC // 2                   # x DMAs split by chunk halves (PE starts at
                               # half-tile arrival; 1 sync-wait per matmul)

# Matmul input dtype:
#   "fp16": half the HBM traffic (the kernel is memory-bound) and full-rate
#           PE; measured absmax rel err ~2.4e-4 vs the fp32 reference.
#   "fp32r": fp32 bits via the fast PE mode; absmax rel err ~1.2e-4.
MM_MODE = "fp16"
MM_DT = mybir.dt.float16 if MM_MODE == "fp16" else mybir.dt.float32r
MM_NP = np.float16 if MM_MODE == "fp16" else np.float32
F32 = mybir.dt.float32


def _build_bass():
    nc = bacc.Bacc("TRN2", target_bir_lowering=False)

    # x pre-tiled on host to [blk, chunk_half, partition, KH*BLK] so each
    # DMA moves one fully-contiguous run per partition.
    xT = nc.dram_tensor(
        "xT", [NBLK, 2, 128, KH * BLK], MM_DT, kind="ExternalInput"
    )
    # w pre-packed on host to the SBUF layout [partition, kchunk*M]:
    # one contiguous run per partition keeps its DMA small and fast.
    wT = nc.dram_tensor("wT", [128, KC * M], MM_DT, kind="ExternalInput")
    nzT = nc.dram_tensor("noiseT", [M, TOK], F32, kind="ExternalInput")
    outT = nc.dram_tensor("outT", [M, TOK], F32, kind="ExternalOutput")

    with tile.TileContext(nc) as tc:
        with (
            tc.tile_pool(name="w", bufs=1) as wpool,
            tc.tile_pool(name="nz", bufs=1) as nzpool,
            tc.tile_pool(name="x", bufs=6) as xpool,
            tc.tile_pool(name="out", bufs=4) as opool,
            tc.tile_pool(name="psum", bufs=NBLK, space="PSUM") as ppool,
        ):
            # Noise on the scalar HWDGE ring (off the x stream's ring).
            nz_tile = nzpool.tile([M, TOK], F32)
            nc.scalar.dma_start(nz_tile[:], nzT[:])

            # First x half-tile, then w, then the rest: the sync ring is
            # FIFO, so w lands right after the first x half and PE can
            # start at ~first-tile time.
            x_tiles = [[None, None] for _ in range(NBLK)]

            def load_x(b, hb):
                t = xpool.tile([128, KH, BLK], MM_DT, tag=f"xt{hb}")
                nc.sync.dma_start(
                    t[:], xT[b, hb].rearrange("p (c t) -> p c t", c=KH)
                )
                x_tiles[b][hb] = t

            load_x(0, 0)

            w_tile = wpool.tile([128, KC, M], MM_DT)
            nc.sync.dma_start(w_tile[:], wT.rearrange("p (c m) -> p c m", c=KC))

            load_x(0, 1)
            for b in range(1, NBLK):
                load_x(b, 0)
                load_x(b, 1)

            # fp32/fp16 matmuls self-load weights and their codegen struct
            # only supports a single sync wait. This warmup matmul absorbs
            # the w-DMA wait into PE program order so every real matmul
            # needs only its x-DMA wait.
            warm = ppool.tile([M, M], F32, tag="ptile")
            nc.tensor.matmul(warm[:], w_tile[:, 0, :], w_tile[:, 0, :])

            for b in range(NBLK):
                ptile = ppool.tile([M, BLK], F32, tag="ptile")
                for c in range(KC):
                    nc.tensor.matmul(
                        ptile[:],
                        w_tile[:, c, :],
                        x_tiles[b][c // KH][:, c % KH, :],
                        start=(c == 0),
                        stop=(c == KC - 1),
                    )

                o_tile = opool.tile([M, BLK], F32)
                nc.vector.tensor_add(
                    o_tile[:], ptile[:], nz_tile[:, b * BLK : (b + 1) * BLK]
                )
                nc.scalar.dma_start(outT[:, b * BLK : (b + 1) * BLK], o_tile[:])

    nc.compile()
    return nc


_NC_CACHE = None


def _get_nc():
    global _NC_CACHE
    if _NC_CACHE is None:
        _NC_CACHE = _build_bass()
    return _NC_CACHE


def _hadamard32() -> np.ndarray:
    h = np.array([[1.0]], dtype=np.float64)
    while h.shape[0] < M:
        h = np.block([[h, h], [h, -h]])
    return h


_NOISE_CACHE = None


def _noise() -> np.ndarray:
    # Mirror reference.py exactly (same op on the default jax backend): the
    # bits differ between backends, so the noise must be produced the same
    # way the grading reference produces it.
    global _NOISE_CACHE
    if _NOISE_CACHE is None:
        import jax

        nz = NOISE_STD * jax.random.normal(
            jax.random.key(42), (B, N, M), dtype=np.float32
        )
        _NOISE_CACHE = np.asarray(nz)
    return _NOISE_CACHE


def kernel(x: np.ndarray, W: np.ndarray, _profile_sink=None) -> np.ndarray:
    x = np.ascontiguousarray(np.asarray(x, dtype=np.float32))
    W = np.asarray(W, dtype=np.float32)

    # Fold normalized FWHT into the projection: out = x @ w_lhsT + noise
    w_eff = (_hadamard32() @ W.astype(np.float64)) / math.sqrt(M)
    w_lhsT = w_eff.T.astype(MM_NP)  # [D, M]
    # pack to device SBUF layout [partition, kchunk, M]
    w_dev = np.ascontiguousarray(
        w_lhsT.reshape(KC, 128, M).transpose(1, 0, 2)
    ).reshape(128, KC * M)

    noise = _noise().reshape(TOK_TOTAL, M)
    X = x.reshape(TOK_TOTAL, D).astype(MM_NP, copy=False)

    in_maps = []
    for i in range(N_CORES):
        sl = slice(i * TOK, (i + 1) * TOK)
        # [tok, d] -> [blk, chunk_half, partition, kchunk, tok] contiguous
        xt = np.ascontiguousarray(
            X[sl].reshape(NBLK, BLK, 2, KH, 128).transpose(0, 2, 4, 3, 1)
        ).reshape(NBLK, 2, 128, KH * BLK)
        in_maps.append(
            {
                "xT": xt,
                "wT": w_dev,
                "noiseT": np.ascontiguousarray(noise[sl].T),
            }
        )

    res = run_bass_kernel_spmd(
        _get_nc(),
        in_maps,
        core_ids=list(range(N_CORES)),
        trace=_profile_sink is not None,
    )
    if _profile_sink is not None:
        _profile_sink.append(res)

    out = np.concatenate([r["outT"].T for r in res.results], axis=0)
    return np.ascontiguousarray(out.reshape(B, N, M).astype(np.float32))


if __name__ == "__main__":
    xs = np.random.randn(B, N, D).astype(np.float32)
    Ws = (np.random.randn(M, D) / math.sqrt(D)).astype(np.float32)
    o = kernel(xs, Ws)
    print(o.shape, o.dtype)


# revision 29
# speedup vs baseline: 1.0375x; 1.0375x over previous
"""Trainium2 Bass kernel for ContinuousREWAEncoder:
    out = FWHT(x @ W^T)/sqrt(32) + 0.01*normal(key=42)

Math folding: FWHT is linear => out = x @ (H @ W / sqrt(32))^T + noise.
The noise uses a fixed PRNG key, so it is a deterministic constant computed
on host (with the same jax op/backend as the reference) and streamed in.

Sharding: pure data parallel over tokens (B*N = 32768 -> 4096/core on 8
cores). W_eff is replicated. Each x shard is pre-tiled on host so the
contraction dim D lies on SBUF partitions and every DMA is one contiguous
run per partition. The device does a single streaming GEMM:
psum[32, t] += Wc[128,32]^T @ xT[128, t] accumulated over 8 d-chunks, with
the noise added during PSUM evacuation by the DVE, and the output stored
transposed [32, tok] (un-transposed on host).
"""

import math

import numpy as np

import concourse.bass as bass
import concourse.tile as tile
from concourse import bacc, mybir
from concourse.bass_utils import run_bass_kernel_spmd

B, N, D, M = 4, 8192, 1024, 32
NOISE_STD = 0.01
N_CORES = 8
TOK_TOTAL = B * N              # 32768
TOK = TOK_TOTAL // N_CORES     # 4096 tokens per core
BLK = 512                      # tokens per PSUM bank ([32, 512] fp32 = 1 bank)
NBLK = TOK // BLK              # 8 -> exactly the 8 PSUM banks
KC = D // 128                  # 8 contraction chunks

# Matmul input dtype:
#   "fp16": half the HBM traffic (the kernel is memory-bound) and full-rate
#           PE; measured absmax rel err ~2.4e-4 vs the fp32 reference.
#   "fp32r": fp32 bits via the fast PE mode; absmax rel err ~1.2e-4.
MM_MODE = "fp16"
MM_DT = mybir.dt.float16 if MM_MODE == "fp16" else mybir.dt.float32r
MM_NP = np.float16 if MM_MODE == "fp16" else np.float32
F32 = mybir.dt.float32


def _build_bass():
    nc = bacc.Bacc("TRN2", target_bir_lowering=False)

    # x pre-tiled on host to [blk, partition, kchunk*BLK] so each DMA moves
    # one fully-contiguous run per partition (128 big descriptors -> full
    # HBM streaming rate).
    xT = nc.dram_tensor("xT", [NBLK, 128, KC * BLK], MM_DT, kind="ExternalInput")
    # w pre-packed on host to the SBUF layout [partition, kchunk*M]:
    # one contiguous run per partition keeps its DMA small and fast.
    wT = nc.dram_tensor("wT", [128, KC * M], MM_DT, kind="ExternalInput")
    nzT = nc.dram_tensor("noiseT", [M, TOK], F32, kind="ExternalInput")
    outT = nc.dram_tensor("outT", [M, TOK], F32, kind="ExternalOutput")

    with tile.TileContext(nc) as tc:
        with (
            tc.tile_pool(name="w", bufs=1) as wpool,
            tc.tile_pool(name="nz", bufs=1) as nzpool,
            tc.tile_pool(name="x", bufs=6) as xpool,
            tc.tile_pool(name="out", bufs=4) as opool,
            tc.tile_pool(name="psum", bufs=NBLK, space="PSUM") as ppool,
        ):
            # Noise on the scalar HWDGE ring (off the x stream's ring).
            nz_tile = nzpool.tile([M, TOK], F32)
            nc.scalar.dma_start(nz_tile[:], nzT[:])

            # w on the sync ring ahead of the x stream (FIFO per ring) so
            # the warmup matmul unblocks before the first x tile lands.
            w_tile = wpool.tile([128, KC, M], MM_DT)
            nc.sync.dma_start(w_tile[:], wT.rearrange("p (c m) -> p c m", c=KC))

            x_tiles = []
            for b in range(NBLK):
                t = xpool.tile([128, KC, BLK], MM_DT, tag="xt")
                nc.sync.dma_start(t[:], xT[b].rearrange("p (c t) -> p c t", c=KC))
                x_tiles.append(t)

            # fp32/fp16 matmuls self-load weights and their codegen struct
            # only supports a single sync wait. This warmup matmul absorbs
            # the w-DMA wait into PE program order so every real matmul
            # needs only its x-DMA wait.
            warm = ppool.tile([M, M], F32, tag="ptile")
            nc.tensor.matmul(warm[:], w_tile[:, 0, :], w_tile[:, 0, :])

            for b in range(NBLK):
                ptile = ppool.tile([M, BLK], F32, tag="ptile")
                for c in range(KC):
                    nc.tensor.matmul(
                        ptile[:],
                        w_tile[:, c, :],
                        x_tiles[b][:, c, :],
                        start=(c == 0),
                        stop=(c == KC - 1),
                    )

                o_tile = opool.tile([M, BLK], F32)
                nc.vector.tensor_add(
                    o_tile[:], ptile[:], nz_tile[:, b * BLK : (b + 1) * BLK]
                )
                nc.scalar.dma_start(outT[:, b * BLK : (b + 1) * BLK], o_tile[:])

    nc.compile()
    return nc


_NC_CACHE = None


def _get_nc():
    global _NC_CACHE
    if _NC_CACHE is None:
        _NC_CACHE = _build_bass()
    return _NC_CACHE


def _hadamard32() -> np.ndarray:
    h = np.array([[1.0]], dtype=np.float64)
    while h.shape[0] < M:
        h = np.block([[h, h], [h, -h]])
    return h


_NOISE_CACHE = None


def _noise() -> np.ndarray:
    # Mirror reference.py exactly (same op on the default jax backend): the
    # bits differ between backends, so the noise must be produced the same
    # way the grading reference produces it.
    global _NOISE_CACHE
    if _NOISE_CACHE is None:
        import jax

        nz = NOISE_STD * jax.random.normal(
            jax.random.key(42), (B, N, M), dtype=np.float32
        )
        _NOISE_CACHE = np.asarray(nz)
    return _NOISE_CACHE


def kernel(x: np.ndarray, W: np.ndarray, _profile_sink=None) -> np.ndarray:
    x = np.ascontiguousarray(np.asarray(x, dtype=np.float32))
    W = np.asarray(W, dtype=np.float32)

    # Fold normalized FWHT into the projection: out = x @ w_lhsT + noise
    w_eff = (_hadamard32() @ W.astype(np.float64)) / math.sqrt(M)
    w_lhsT = w_eff.T.astype(MM_NP)  # [D, M]
    # pack to device SBUF layout [partition, kchunk, M]
    w_dev = np.ascontiguousarray(
        w_lhsT.reshape(KC, 128, M).transpose(1, 0, 2)
    ).reshape(128, KC * M)

    noise = _noise().reshape(TOK_TOTAL, M)
    X = x.reshape(TOK_TOTAL, D).astype(MM_NP, copy=False)

    in_maps = []
    for i in range(N_CORES):
        sl = slice(i * TOK, (i + 1) * TOK)
        # [tok, d] -> [blk, partition, kchunk, tok_in_blk] contiguous
        xt = np.ascontiguousarray(
            X[sl].reshape(NBLK, BLK, KC, 128).transpose(0, 3, 2, 1)
        ).reshape(NBLK, 128, KC * BLK)
        in_maps.append(
            {
                "xT": xt,
                "wT": w_dev,
                "noiseT": np.ascontiguousarray(noise[sl].T),
            }
        )

    res = run_bass_kernel_spmd(
        _get_nc(),
        in_maps,
        core_ids=list(range(N_CORES)),
        trace=_profile_sink is not None,
    )
    if _profile_sink is not None:
        _profile_sink.append(res)

    out = np.concatenate([r["outT"].T for r in res.results], axis=0)
    return np.ascontiguousarray(out.reshape(B, N, M).astype(np.float32))


if __name__ == "__main__":
    xs = np.random.randn(B, N, D).astype(np.float32)
    Ws = (np.random.randn(M, D) / math.sqrt(D)).astype(np.float32)
    o = kernel(xs, Ws)
    print(o.shape, o.dtype)
